# revision 46
# baseline (speedup 1.0000x reference)
"""3-layer GAT on 8 Trainium2 NeuronCores (Bass/Tile) — v7.

Same math as the v2 baseline (1D node-parallel, deferred-softmax GAT).
The measured warm-call wall under axon is transfer-dominated (~73ms RPC
floor + ~11ms/MB on the tunneled link + ~19ms device exec), so v3-v7
attack the host->device path as much as the device program:

  - Persistent jit runner: the shard_map'd bass_exec executable is built
    once and reused across calls (v2 re-jit'd per call via
    run_bass_kernel_spmd, re-loading the NEFF each time: ~50ms/call).
  - Wire diet (19.7MB -> 14.7MB): the dstw8 edge table is dropped (the
    int16 dst gather index is derived on device from dstf8 via a
    wrap-conversion shuffle done with 16-partition DMAs + max(.,0));
    the weight/bias blob is uploaded sharded 1/8-per-core and
    AllGather'd on device instead of 8x-replicated host-side; x is
    quantized to 7 bits (8 values packed in 7 bytes, MSB-folded,
    per-row scale max/63; unpacked on DVE with bitwise_and + is_lt).
    Final max rel err ~1.1e-2 vs the 2e-2 gate.
  - Self-loops (PyG add_self_loops) are computed analytically in the
    normalize/final phases from local h_own rows instead of being
    materialized as edges: smaller tables and NP 1408 -> 1280.
  - Edges are sorted by src within each dst window (order is free):
    ascending HBM rows for the gather, compressible idx table.
  - UNROLL=2 on the hw loops: with a persistent executable, program size
    is a one-time cost, so double-buffered loop bodies are affordable.

Per layer: transform (h|s|d = z @ Wext, node tiles via hw loop) -> AllGather
of the padded node table -> edge phase (hw loop over 49 dst windows: gather
src rows + dst coefficients, ee = exp(leaky_relu(s+d)), messages = h*ee,
one-hot matmul aggregation into PSUM, PSUM -> acc DRAM) -> normalize
(self-loop term + out = num/den + b, ELU, transpose back into zT).

Known-even alternative: replacing the per-edge dst-coefficient gather
with sel^T (DMA-transpose) x d_win matmuls measured identical wall time;
kept the simpler gather form.
"""

import sys

import numpy as np

sys.path.insert(0, '/opt/trn_rl_repo')

from contextlib import ExitStack

import ml_dtypes

import jax  # noqa: E402

try:
    jax.config.update('jax_compilation_cache_dir', '/tmp/jax_comp_cache')
    jax.config.update('jax_persistent_cache_min_compile_time_secs', 0)
    jax.config.update('jax_persistent_cache_min_entry_size_bytes', 0)
except Exception:
    pass

from jax.sharding import Mesh, PartitionSpec  # noqa: E402
from jax.experimental.shard_map import shard_map  # noqa: E402

from concourse import bacc, mybir, tile  # noqa: E402
from concourse import bass2jax  # noqa: E402
from concourse.bass import ds, ts  # noqa: E402
from concourse.bass2jax import _bass_exec_p, partition_id_tensor  # noqa: E402
from concourse.masks import make_identity  # noqa: E402

F32 = mybir.dt.float32
BF16 = mybir.dt.bfloat16
I16 = mybir.dt.int16
AF = mybir.ActivationFunctionType
ALU = mybir.AluOpType
BF = ml_dtypes.bfloat16


class Cfg:
    def __init__(self, N=50000, F=256, H=8, C=32, OUT=2, NCORES=8, NP=1280,
                 GS=768, UNROLL=2, HWLOOP=True, WAG=True, XBITS=7):
        self.N, self.F, self.H, self.C, self.OUT = N, F, H, C, OUT
        self.NCORES = NCORES
        assert N % NCORES == 0 and F % 128 == 0
        self.KH = F // 128
        self.NPC = N // NCORES                 # real nodes per core
        self.NT = (self.NPC + 127) // 128      # node tiles / edge windows
        self.NPCP = self.NT * 128              # padded nodes per core
        self.NG = NCORES * self.NPCP           # padded global nodes
        self.SPLIT = (NCORES // 2) * self.NPCP  # stream boundary (int16 idx)
        assert self.SPLIT <= 32768 and self.NG - self.SPLIT <= 32768
        self.EW = 384                          # l1-2 row elems (768B, bf16)
        self.EW3 = 128                         # l3 row elems (256B)
        self.WC = F + 2 * H                    # transform cols (h|s|d)
        self.WC3 = OUT + 2
        self.MC = F + H                        # aggregated cols (msg|ee)
        self.MC3 = OUT + 1
        self.NP = NP                           # padded edges per (stream,win)
        assert NP % 128 == 0
        self.GS = GS                           # idxs per dma_gather call
        assert GS % 128 == 0
        self.UNROLL = UNROLL
        self.HWLOOP = HWLOOP
        self.WAG = WAG
        # weight blob: W1e|W2e|W3e (bf16) + b1|b2|b3 (f32), sharded 1/8
        self.WLEN = F * (2 * self.WC + self.WC3)      # i16 units
        self.BLEN = 2 * (2 * F + OUT)                 # i16 units (f32 biases)
        self.WSLI = ((self.WLEN + self.BLEN + 8 * 256 - 1)
                     // (8 * 256)) * 256              # per-core slice, i16
        self.WREG = self.WSLI if WAG else 8 * self.WSLI  # ei region size
        # x quantization: 8 = plain int8; 7 = 7-bit (8 vals packed in 7
        # bytes, MSB-folded; per-row scale max/63). 7-bit keeps the final
        # max rel err ~1.2e-2 (<2e-2 gate) and cuts 1.56MB off the wire.
        self.XBITS = XBITS
        assert XBITS in (7, 8) and self.NPCP % 8 == 0
        if XBITS == 7:
            self.XLEN = F * (self.NPCP // 8) * 7 // 2    # i16 units
        else:
            self.XLEN = F * self.NPC // 2


def _gather_chunks(cfg):
    """[(idx_off, n_idx), ...] covering NP in <=GS multiples of 128."""
    out, off = [], 0
    while off < cfg.NP:
        n = min(cfg.GS, cfg.NP - off)
        out.append((off, n))
        off += n
    return out


def _amat(att):
    Hh, Cc = att.shape
    A = np.zeros((Hh * Cc, Hh), np.float32)
    for h in range(Hh):
        A[h * Cc:(h + 1) * Cc, h] = att[h]
    return A


def _ext_w(W, a_s, a_d):
    Ws = (W @ _amat(a_s)).astype(np.float32)
    Wd = (W @ _amat(a_d)).astype(np.float32)
    return np.ascontiguousarray(
        np.concatenate([W, Ws, Wd], axis=1)).astype(BF)


def _wrap16(arr):
    return np.ascontiguousarray(arr.reshape(-1, 16).T.astype(np.int16))


def _prepare_edges(cfg, src, dst):
    """Host-side partitioning. Returns (NP, per_core list of dicts).

    Per (core, stream): src idx table (int16, 16-wrapped) and dstf table
    (int8, 128-wrapped; window-local dst row 0..127, -1 for pad slots).
    The device derives the int16 dst gather index as max(dstf, 0) via a
    wrap-conversion shuffle, so pads gather (finite) row 0 while their
    one-hot column stays all-zero.
    """
    NC, NPC, NPCP, NT = cfg.NCORES, cfg.NPC, cfg.NPCP, cfg.NT
    core_of = dst // NPC
    dstl = dst - core_of * NPC             # 0..NPC-1 (== padded local row)
    win = dstl // 128
    dstw = dstl - win * 128
    srcp = (src // NPC) * NPCP + (src % NPC)   # padded global row
    stream = (srcp >= cfg.SPLIT).astype(np.int64)

    counts = np.zeros((NC, 2, NT), np.int64)
    np.add.at(counts, (core_of, stream, win), 1)
    NP = int(((max(counts.max(), 1) + 127) // 128) * 128)
    cfg.NP = NP
    T = NT * NP

    per_core = []
    for c in range(NC):
        out = {}
        for s, tag in ((0, 'A'), (1, 'B')):
            src_a = np.zeros(T, np.int64)
            dstf_a = -np.ones(T, np.float32)
            m = (core_of == c) & (stream == s)
            ww = win[m]
            sp = srcp[m]
            # secondary sort by src: edge order within a window is free
            # (one-hot aggregation commutes), and sorted src makes the
            # int16 table compressible for the wire and the hg gather
            # walk HBM rows in ascending order.
            order = np.lexsort((sp, ww))
            ww = ww[order]
            ss = sp[order] - s * cfg.SPLIT
            fw = dstw[m][order]
            grp_start = np.searchsorted(ww, np.arange(NT))
            rank = np.arange(len(ww)) - grp_start[ww]
            pos = ww * NP + rank
            src_a[pos] = ss
            dstf_a[pos] = fw
            out['src' + tag] = _wrap16(src_a)
            out['dstf8' + tag] = np.ascontiguousarray(
                dstf_a.reshape(-1, 128).T.astype(np.int8))
        blob = np.concatenate([
            out['srcA'].ravel(), out['srcB'].ravel(),
            np.concatenate([out['dstf8A'].ravel(),
                            out['dstf8B'].ravel()]).view(np.int16)])
        per_core.append({'ei': np.ascontiguousarray(blob)[None, :]})
    return NP, per_core


def _build(cfg, abl=''):
    # abl: debug-only ablation for timing probes ('' = full kernel):
    #   'noedge' - skip edge phases; 'nodg' - skip dst-coeff gathers;
    #   'gonly'  - gathers without edge compute; 'noag' - no collectives.
    NC, NT, NP, KH = cfg.NCORES, cfg.NT, cfg.NP, cfg.KH
    F, H, C, OUT = cfg.F, cfg.H, cfg.C, cfg.OUT
    NPC, NPCP, NG = cfg.NPC, cfg.NPCP, cfg.NG
    EW, EW3, WC, WC3, MC, MC3 = (cfg.EW, cfg.EW3, cfg.WC, cfg.WC3, cfg.MC,
                                 cfg.MC3)
    T16 = NT * NP // 16
    T128 = NT * NP // 128
    NS = NP // 128                      # 128-edge slices per (stream, window)
    chunks = _gather_chunks(cfg)

    nc = bacc.Bacc('TRN2', target_bir_lowering=False, debug=False,
                   num_devices=NC)

    # ---- I/O ----
    T = NT * NP
    # ALL inputs are packed into ONE int16 tensor (bitcast views carve out
    # int8/bf16/f32 regions): PJRT-over-axon charges a fixed cost per input
    # array per call, so array count matters as much as bytes.
    # Layout (i16 units): srcA|srcB (16-wrapped int16), dstf8A|dstf8B
    # (128-wrapped int8 window offsets, -1 pads), xT (int8 pairs),
    # wsl (per-core 1/8 slice of W1e|W2e|W3e bf16 + b1|b2|b3 f32),
    # fb (f32 x row scales).
    XOFF = 3 * T
    XLEN = cfg.XLEN
    WOFF = XOFF + XLEN
    WSLI = cfg.WSLI
    FOFF = WOFF + cfg.WREG
    NF32 = 128 * NT
    assert XOFF % 2 == 0 and FOFF % 2 == 0
    ei = nc.dram_tensor('ei', [1, FOFF + 2 * NF32], I16,
                        kind='ExternalInput')
    xT = ei[0:1, XOFF:XOFF + XLEN].bitcast(mybir.dt.int8).rearrange(
        'x (p c) -> (x p) c', p=F)
    fb = ei[0:1, FOFF:FOFF + 2 * NF32].bitcast(F32)
    out_own = nc.dram_tensor('out_own', [NPCP, OUT], mybir.dt.float16,
                             kind='ExternalOutput')

    # ---- internal DRAM ----
    h_own = nc.dram_tensor('h_own', [NPCP, EW], BF16)
    h_full = nc.dram_tensor('h_full', [NG, EW], BF16, addr_space='Shared')
    h3_own = nc.dram_tensor('h3_own', [NPCP, EW3], BF16)
    h3_full = nc.dram_tensor('h3_full', [NG, EW3], BF16, addr_space='Shared')
    acc_d = nc.dram_tensor('acc', [NPCP, MC], F32)
    acc3_d = nc.dram_tensor('acc3', [NPCP, MC3], F32)
    if cfg.WAG:
        wsl_d = nc.dram_tensor('wsl', [128, WSLI // 128], I16)
        wfull_d = nc.dram_tensor('wfull', [NC * 128, WSLI // 128], I16,
                                 addr_space='Shared')

    with tile.TileContext(nc) as tc, ExitStack() as ctx:
        const = ctx.enter_context(tc.tile_pool(name='const', bufs=1))
        sb = ctx.enter_context(tc.tile_pool(name='sb', bufs=2))
        eb = ctx.enter_context(tc.tile_pool(name='eb', bufs=2))
        ps = ctx.enter_context(tc.tile_pool(name='ps', bufs=2, space='PSUM'))
        ps1 = ctx.enter_context(tc.tile_pool(name='ps1', bufs=1,
                                             space='PSUM'))

        # ---- weight allgather: 1/8 slice -> full blob on every core ----
        if cfg.WAG:
            with tc.tile_pool(name='wstg', bufs=1) as wstg:
                wt_s = wstg.tile([128, WSLI // 128], I16, tag='wsl')
                nc.sync.dma_start(
                    out=wt_s[:],
                    in_=ei[0:1, WOFF:WOFF + WSLI].rearrange(
                        'x (p c) -> (x p) c', p=128))
                nc.sync.dma_start(out=wsl_d[:, :], in_=wt_s[:])
            nc.gpsimd.collective_compute(
                'AllGather', ALU.bypass, ins=[wsl_d[:, :]],
                outs=[wfull_d[:, :]], replica_groups=[list(range(NC))])
            wflat = wfull_d[:, :].rearrange('(x p) c -> x (p c)', x=1)
        else:
            wflat = ei[0:1, WOFF:WOFF + 8 * WSLI]
        wb = wflat[0:1, 0:cfg.WLEN].bitcast(BF16).rearrange(
            'x (p c) -> (x p) c', p=F)
        fbw = wflat[0:1, cfg.WLEN:cfg.WLEN + cfg.BLEN].bitcast(F32)

        # ---- constants / resident tables ----
        iota_t = const.tile([128, 128], BF16)
        nc.gpsimd.iota(iota_t[:], pattern=[[1, 128]], base=0,
                       channel_multiplier=0,
                       allow_small_or_imprecise_dtypes=True)
        ident = const.tile([128, 128], F32)
        make_identity(nc, ident[:])
        b_t = {}
        for name, off, w in (('b1', 0, F), ('b2', F, F), ('b3', 2 * F, OUT)):
            t = const.tile([128, w], F32, tag='b_' + name)
            nc.sync.dma_start(out=t[:],
                              in_=fbw[:, off:off + w].to_broadcast((128, w)))
            b_t[name] = t
        w_t = {}
        for name, off, w in (('W1', 0, WC), ('W2', WC, WC),
                             ('W3', 2 * WC, WC3)):
            t = const.tile([128, KH, w], BF16, tag='w_' + name)
            for kh in range(KH):
                nc.sync.dma_start(out=t[:, kh, :],
                                  in_=wb[kh * 128:(kh + 1) * 128,
                                         off:off + w])
            w_t[name] = t
        idx_t = {}
        for bi, tag in ((0, 'A'), (1, 'B')):
            t = const.tile([128, T16], I16, tag='src' + tag)
            view = ei[0:1, bi * T:(bi + 1) * T].rearrange(
                'x (p c) -> (x p) c', p=16)
            for k in range(8):
                nc.sync.dma_start(out=t[16 * k:16 * (k + 1), :], in_=view)
            idx_t['src' + tag] = t
        ei8 = ei[0:1, 2 * T:3 * T].bitcast(mybir.dt.int8)   # [1, 2T] bytes
        with tc.tile_pool(name='stg', bufs=1) as stg:
            st = stg.tile([128, T128], mybir.dt.int8, tag='stg8')
            for si, tag in ((0, 'A'), (1, 'B')):
                nc.sync.dma_start(
                    out=st[:, 0:T128],
                    in_=ei8[0:1, si * T:(si + 1) * T].rearrange(
                        'x (p c) -> (x p) c', p=128))
                tf = const.tile([128, T128], BF16, tag='dstf' + tag)
                nc.vector.tensor_copy(out=tf[:], in_=st[:, 0:T128])
                idx_t['dstf' + tag] = tf
                # derive the 16-wrapped int16 dst gather index from the
                # 128-wrapped bytes: idx16[16a+r, c] = dstf[(c%8)*16+r, c//8].
                # Convert+clamp full-width on DVE (partition starts must be
                # 0/32/64/96), then shuffle with 16-partition SBUF DMAs.
                sti = stg.tile([128, T128], I16, tag='stg16')
                nc.vector.tensor_copy(out=sti[:], in_=st[:, 0:T128])
                nc.vector.tensor_scalar_max(sti[:], sti[:], 0)
                ti = const.tile([128, T16], I16, tag='dst' + tag)
                tiv = ti.rearrange('p (c m) -> p c m', m=8)
                for a in range(8):
                    for m in range(8):
                        nc.sync.dma_start(
                            out=tiv[16 * a:16 * (a + 1), :, m],
                            in_=sti[16 * m:16 * (m + 1), :])
                idx_t['dst' + tag] = ti
        zT = const.tile([128, KH, NPCP], BF16)
        xs_t = const.tile([128, NT], F32)
        nc.sync.dma_start(out=xs_t[:], in_=fb[:, 0:NF32].rearrange(
            'x (p t) -> (x p) t', p=128))

        def loop(n, body, unroll):
            if cfg.HWLOOP and n > unroll:
                tc.For_i_unrolled(0, n, 1, body, max_unroll=unroll)
            else:
                for i in range(n):
                    body(i)

        # ---- phase bodies ----
        def transform_body(i, wt, wc, hout, ew, scaled=False):
            pool = ps if wc > 16 else ps1
            p = pool.tile([128, wc], F32, tag=f'ps_tr{wc}')
            # walrus can't take register offsets on the ldweights operand, so
            # stage the dynamic zT slice into a static tile first.
            zs = sb.tile([128, KH, 128], BF16, tag='sb_zs')
            nc.vector.tensor_copy(out=zs[:], in_=zT[:, :, ts(i, 128)])
            for kh in range(KH):
                nc.tensor.matmul(p[:, :], lhsT=zs[:, kh, :],
                                 rhs=wt[:, kh, :], start=(kh == 0),
                                 stop=(kh == KH - 1))
            ht = sb.tile([128, wc], BF16, tag=f'sb_tr{wc}')
            if scaled:
                # undo the int8 row quantization of x (scale is per node,
                # nodes sit on partitions here)
                nc.vector.tensor_tensor(
                    out=ht[:], in0=p[:],
                    in1=xs_t[:, ts(i, 1)].to_broadcast((128, wc)),
                    op=ALU.mult)
            else:
                nc.vector.tensor_copy(out=ht[:], in_=p[:])
            nc.sync.dma_start(out=hout[ts(i, 128), 0:wc], in_=ht[:])

        def edge_body(w, tblf, dtbl, dcol, ew, mc, hh, cc, s_off, d_off,
                      acc):
            """One destination window: gather, ee, messages, aggregate."""
            pool = ps if mc > 16 else ps1
            mm = []  # (lhsT slice, rhs slice) accumulation chain
            for si, (s, tag) in enumerate(((0, 'A'), (1, 'B'))):
                tbl = (tblf[0:cfg.SPLIT, :] if s == 0 else
                       tblf[cfg.SPLIT:NG, :])
                # one set of edge tiles serves all layers: layer 3 (ew=128)
                # reinterprets the 384-wide rows as 3x128 slices
                hg_t = eb.tile([128, NS, EW], BF16, tag=f'hg{tag}')
                hg = (hg_t if ew == EW else
                      hg_t.rearrange('p n (k e) -> p (n k) e',
                                     k=EW // ew)[:, 0:NS, :])
                dg = eb.tile([128, NS, 128], BF16, tag=f'dg{tag}')
                for off, ni in chunks:
                    i16 = w * (NP // 16) + off // 16
                    sl = off // 128
                    nc.gpsimd.dma_gather(
                        hg[:, sl:sl + ni // 128, :], tbl,
                        idx_t['src' + tag][:, ds(i16, ni // 16)],
                        num_idxs=ni, num_idxs_reg=ni, elem_size=ew)
                    if abl != 'nodg':
                        nc.gpsimd.dma_gather(
                            dg[:, sl:sl + ni // 128, :],
                            dtbl[ds(w * 128, 128), dcol:dcol + 128],
                            idx_t['dst' + tag][:, ds(i16, ni // 16)],
                            num_idxs=ni, num_idxs_reg=ni, elem_size=128,
                            elem_step=ew)
                if abl == 'nodg':
                    nc.vector.memset(dg[:], 0)
                if abl == 'gonly':
                    gc = sb.tile([128, 1], F32, tag=f'gonly_c{tag}')
                    nc.vector.tensor_tensor(out=gc[:], in0=hg[:, 0, 0:1],
                                            in1=dg[:, 0, 0:1], op=ALU.add)
                    nc.sync.dma_start(
                        out=acc[ts(w, 128), si:si + 1], in_=gc[:])
                    continue
                e8_t = eb.tile([128, NS, H], F32, tag=f'e8{tag}')
                e8 = e8_t[:, :, 0:hh]
                nc.vector.tensor_tensor(
                    out=e8[:], in0=hg[:, :, s_off:s_off + hh],
                    in1=dg[:, :, d_off:d_off + hh], op=ALU.add)
                el_t = eb.tile([128, NS, H], F32, tag=f'el{tag}')
                el = el_t[:, :, 0:hh]
                nc.vector.tensor_scalar_mul(el[:], e8[:], 0.2)
                nc.vector.tensor_tensor(out=el[:], in0=el[:], in1=e8[:],
                                        op=ALU.max)
                msg_t = eb.tile([128, NS, MC], BF16, tag=f'msg{tag}')
                msg = msg_t[:, :, 0:mc]
                nc.scalar.activation(msg[:, :, hh * cc:hh * cc + hh], el[:],
                                     AF.Exp)
                nc.vector.tensor_tensor(
                    out=msg[:, :, 0:hh * cc].rearrange(
                        'p n (h c) -> p n h c', h=hh),
                    in0=hg[:, :, 0:hh * cc].rearrange(
                        'p n (h c) -> p n h c', h=hh),
                    in1=msg[:, :, hh * cc:hh * cc + hh][
                        :, :, :, None].to_broadcast((128, NS, hh, cc)),
                    op=ALU.mult)
                sel = eb.tile([128, NS, 128], BF16, tag=f'sel{tag}')
                nc.vector.tensor_tensor(
                    out=sel[:],
                    in0=iota_t[:, None, :].to_broadcast((128, NS, 128)),
                    in1=idx_t['dstf' + tag][:, ts(w, NS), None].to_broadcast(
                        (128, NS, 128)),
                    op=ALU.is_equal)
                for j in range(NS):
                    mm.append((sel[:, j, :], msg[:, j, :]))
            if abl == 'gonly':
                return
            p = pool.tile([128, mc], F32, tag=f'ps_agg{mc}')
            for j, (lh, rh) in enumerate(mm):
                nc.tensor.matmul(p[:, :], lhsT=lh, rhs=rh, start=(j == 0),
                                 stop=(j == len(mm) - 1))
            ac = sb.tile([128, mc], F32, tag=f'ac{mc}')
            nc.vector.tensor_copy(out=ac[:], in_=p[:])
            nc.sync.dma_start(out=acc[ts(w, 128), :], in_=ac[:])

        def normalize_body(i, bt):
            a = sb.tile([128, MC], F32, tag='nrm_a')
            nc.sync.dma_start(out=a[:], in_=acc_d[ts(i, 128), :])
            # self-loop contribution: ee = exp(lrelu(s_i + d_i)),
            # num += h_i * ee, den += ee (rows are local, no gather needed)
            hh = sb.tile([128, WC], BF16, tag='nrm_hh')
            nc.sync.dma_start(out=hh[:], in_=h_own[ts(i, 128), 0:WC])
            tsd = sb.tile([128, H], F32, tag='nrm_t')
            nc.vector.tensor_tensor(out=tsd[:], in0=hh[:, F:F + H],
                                    in1=hh[:, F + H:F + 2 * H], op=ALU.add)
            tl = sb.tile([128, H], F32, tag='nrm_tl')
            nc.vector.tensor_scalar_mul(tl[:], tsd[:], 0.2)
            nc.vector.tensor_tensor(out=tl[:], in0=tl[:], in1=tsd[:],
                                    op=ALU.max)
            es = sb.tile([128, H], F32, tag='nrm_es')
            nc.scalar.activation(es[:], tl[:], AF.Exp)
            nc.vector.tensor_tensor(out=a[:, F:F + H], in0=a[:, F:F + H],
                                    in1=es[:], op=ALU.add)
            hm = sb.tile([128, F], F32, tag='nrm_hm')
            nc.vector.tensor_tensor(
                out=hm[:].rearrange('p (h c) -> p h c', h=H),
                in0=hh[:, 0:F].rearrange('p (h c) -> p h c', h=H),
                in1=es[:, :, None].to_broadcast((128, H, C)), op=ALU.mult)
            nc.vector.tensor_tensor(out=a[:, 0:F], in0=a[:, 0:F], in1=hm[:],
                                    op=ALU.add)
            r = sb.tile([128, H], F32, tag='nrm_r')
            nc.vector.tensor_scalar_add(r[:], a[:, F:F + H], 1e-16)
            rr = sb.tile([128, H], F32, tag='nrm_rr')
            nc.vector.reciprocal(rr[:], r[:])
            z = sb.tile([128, F], F32, tag='nrm_z')
            nc.vector.tensor_tensor(
                out=z[:].rearrange('p (h c) -> p h c', h=H),
                in0=a[:, 0:F].rearrange('p (h c) -> p h c', h=H),
                in1=rr[:, :, None].to_broadcast((128, H, C)), op=ALU.mult)
            nc.vector.tensor_tensor(out=z[:], in0=z[:], in1=bt[:],
                                    op=ALU.add)
            # ELU: max(z,0) + exp(min(z,0)) - 1
            zn = sb.tile([128, F], F32, tag='nrm_zn')
            nc.vector.tensor_scalar_min(zn[:], z[:], 0.0)
            en = sb.tile([128, F], F32, tag='nrm_en')
            nc.scalar.activation(en[:], zn[:], AF.Exp)
            nc.vector.tensor_scalar_add(en[:], en[:], -1.0)
            nc.vector.tensor_scalar_max(z[:], z[:], 0.0)
            nc.vector.tensor_tensor(out=z[:], in0=z[:], in1=en[:], op=ALU.add)
            for kh in range(KH):
                tp = ps1.tile([128, 128], F32, tag='ps_tp')
                nc.tensor.transpose(out=tp[:, :],
                                    in_=z[:, kh * 128:(kh + 1) * 128],
                                    identity=ident[:, :])
                nc.vector.tensor_copy(out=zT[:, kh, ts(i, 128)], in_=tp[:, :])

        def final_body(i):
            a = sb.tile([128, MC3], F32, tag='fo_a')
            nc.sync.dma_start(out=a[:], in_=acc3_d[ts(i, 128), :])
            hh = sb.tile([128, WC3], BF16, tag='fo_hh')
            nc.sync.dma_start(out=hh[:], in_=h3_own[ts(i, 128), 0:WC3])
            tsd = sb.tile([128, 1], F32, tag='fo_t')
            nc.vector.tensor_tensor(out=tsd[:], in0=hh[:, OUT:OUT + 1],
                                    in1=hh[:, OUT + 1:OUT + 2], op=ALU.add)
            tl = sb.tile([128, 1], F32, tag='fo_tl')
            nc.vector.tensor_scalar_mul(tl[:], tsd[:], 0.2)
            nc.vector.tensor_tensor(out=tl[:], in0=tl[:], in1=tsd[:],
                                    op=ALU.max)
            es = sb.tile([128, 1], F32, tag='fo_es')
            nc.scalar.activation(es[:], tl[:], AF.Exp)
            nc.vector.tensor_tensor(out=a[:, OUT:OUT + 1],
                                    in0=a[:, OUT:OUT + 1], in1=es[:],
                                    op=ALU.add)
            hm = sb.tile([128, OUT], F32, tag='fo_hm')
            nc.vector.tensor_tensor(out=hm[:], in0=hh[:, 0:OUT],
                                    in1=es[:, :].to_broadcast((128, OUT)),
                                    op=ALU.mult)
            nc.vector.tensor_tensor(out=a[:, 0:OUT], in0=a[:, 0:OUT],
                                    in1=hm[:], op=ALU.add)
            r = sb.tile([128, 1], F32, tag='fo_r')
            nc.vector.tensor_scalar_add(r[:], a[:, OUT:OUT + 1], 1e-16)
            rr = sb.tile([128, 1], F32, tag='fo_rr')
            nc.vector.reciprocal(rr[:], r[:])
            o = sb.tile([128, OUT], F32, tag='fo_o')
            nc.vector.tensor_tensor(out=o[:], in0=a[:, 0:OUT],
                                    in1=rr[:, :].to_broadcast((128, OUT)),
                                    op=ALU.mult)
            o16 = sb.tile([128, OUT], mybir.dt.float16, tag='fo_o16')
            nc.vector.tensor_tensor(out=o16[:], in0=o[:], in1=b_t['b3'][:],
                                    op=ALU.add)
            nc.sync.dma_start(out=out_own[ts(i, 128), :], in_=o16[:])

        def allgather(src_d, dst_d):
            nc.gpsimd.collective_compute(
                'AllGather', ALU.bypass, ins=[src_d[:, :]], outs=[dst_d[:, :]],
                replica_groups=[list(range(NC))])

        # ---------------- program ----------------
        U = cfg.UNROLL
        with nc.named_scope('tr1'):
            # Zero-fill node tables once: the pad columns (WC:EW) are never
            # consumed, but stale DRAM would trip the sim's finite check.
            zpad = const.tile([128, EW], BF16)
            nc.vector.memset(zpad[:], 0)
            nc.sync.dma_start(
                out=h_own[:, :].rearrange('(t p) e -> p t e', p=128),
                in_=zpad[:, None, :].to_broadcast((128, NT, EW)))
            nc.sync.dma_start(
                out=h3_own[:, :].rearrange('(t p) e -> p t e', p=128),
                in_=zpad[:, None, 0:EW3].to_broadcast((128, NT, EW3)))
            if abl == 'noedge':
                zp32 = const.tile([128, MC], F32, tag='ablz')
                nc.vector.memset(zp32[:], 0)
                nc.sync.dma_start(
                    out=acc_d[:, :].rearrange('(t p) e -> p t e', p=128),
                    in_=zp32[:, None, 0:MC].to_broadcast((128, NT, MC)))
                nc.sync.dma_start(
                    out=acc3_d[:, :].rearrange('(t p) e -> p t e', p=128),
                    in_=zp32[:, None, 0:MC3].to_broadcast((128, NT, MC3)))
            if abl == 'noag':
                nc.sync.dma_start(
                    out=h_full[:, :].rearrange('(t p) e -> p t e', p=128),
                    in_=zpad[:, None, :].to_broadcast((128, NT * NC, EW)))
                nc.sync.dma_start(
                    out=h3_full[:, :].rearrange('(t p) e -> p t e', p=128),
                    in_=zpad[:, None, 0:EW3].to_broadcast(
                        (128, NT * NC, EW3)))
            if cfg.XBITS == 7:
                # unpack: 8 values per 7 bytes; b_i holds u_i (low 7 bits)
                # and bit i of u_7 (MSB). v = u - 64.
                XB = NPCP // 8 * 7
                xp = const.tile([128, KH, XB], mybir.dt.int8)
                for kh in range(KH):
                    nc.sync.dma_start(out=xp[:, kh, :],
                                      in_=xT[kh * 128:(kh + 1) * 128, :])
                xu = const.tile([128, KH, NPCP], mybir.dt.int8)
                xpv = xp.rearrange('p k (g i) -> p k g i', i=7)
                xuv = xu.rearrange('p k (g i) -> p k g i', i=8)
                nc.vector.tensor_scalar(
                    out=xuv[:, :, :, 0:7], in0=xpv[:], scalar1=127,
                    scalar2=None, op0=ALU.bitwise_and)
                a7 = const.tile([128, KH, NPCP // 8], mybir.dt.int8,
                                tag='x7a')
                t7 = const.tile([128, KH, NPCP // 8], mybir.dt.int8,
                                tag='x7t')
                for i in range(7):
                    dstt = a7 if i == 0 else t7
                    nc.vector.tensor_scalar(
                        out=dstt[:], in0=xpv[:, :, :, i], scalar1=0,
                        scalar2=1 << i, op0=ALU.is_lt, op1=ALU.mult)
                    if i:
                        nc.vector.tensor_tensor(out=a7[:], in0=a7[:],
                                                in1=t7[:], op=ALU.add)
                nc.vector.tensor_copy(out=xuv[:, :, :, 7], in_=a7[:])
                nc.vector.tensor_scalar_add(zT[:, :, 0:NPCP], xu[:], -64.0)
            else:
                xi8 = const.tile([128, KH, NPC], mybir.dt.int8)
                for kh in range(KH):
                    nc.sync.dma_start(out=xi8[:, kh, :],
                                      in_=xT[kh * 128:(kh + 1) * 128, :])
                nc.vector.tensor_copy(out=zT[:, :, 0:NPC], in_=xi8[:])
                if NPCP > NPC:
                    nc.vector.memset(zT[:, :, NPC:NPCP], 0)
            loop(NT, lambda i: transform_body(i, w_t['W1'], WC, h_own, EW,
                                              scaled=True), U)
        with nc.named_scope('ag1'):
            if abl != 'noag':
                allgather(h_own, h_full)
        with nc.named_scope('edges1'):
            if abl != 'noedge':
                loop(NT, lambda w: edge_body(
                    w, h_full, h_own, F, EW, MC, H, C, F, H, acc_d), U)
        with nc.named_scope('tr2'):
            loop(NT, lambda i: normalize_body(i, b_t['b1']), U)
            loop(NT, lambda i: transform_body(i, w_t['W2'], WC, h_own, EW), U)
        with nc.named_scope('ag2'):
            if abl != 'noag':
                allgather(h_own, h_full)
        with nc.named_scope('edges2'):
            if abl != 'noedge':
                loop(NT, lambda w: edge_body(
                    w, h_full, h_own, F, EW, MC, H, C, F, H, acc_d), U)
        with nc.named_scope('tr3'):
            loop(NT, lambda i: normalize_body(i, b_t['b2']), U)
            loop(NT, lambda i: transform_body(i, w_t['W3'], WC3, h3_own, EW3),
                 U)
        with nc.named_scope('ag3'):
            if abl != 'noag':
                allgather(h3_own, h3_full)
        with nc.named_scope('edges3'):
            if abl != 'noedge':
                loop(NT, lambda w: edge_body(
                    w, h3_full, h3_own, 0, EW3, MC3, 1, OUT, OUT, OUT + 1,
                    acc3_d), U)
        with nc.named_scope('fin'):
            loop(NT, final_body, U)

    nc.compile()
    # The module is immutable from here on; memoize its serialization so the
    # per-call jax lowering doesn't redo ~20ms of json+zstd work each run.
    _json = nc.to_json_bytes()
    nc.to_json_bytes = lambda: _json
    return nc


class _Runner:
    """Persistent shard_map'd bass_exec executable (the axon redirect path
    of run_bass_kernel_spmd, with the jit built ONCE and reused so the NEFF
    stays loaded across calls)."""

    def __init__(self, nc, ncores):
        bass2jax.install_neuronx_cc_hook()
        self.nc, self.ncores = nc, ncores
        partition_name = (nc.partition_id_tensor.name
                          if nc.partition_id_tensor else None)
        in_names, out_names, out_avals, zero_outs = [], [], [], []
        for alloc in nc.m.functions[0].allocations:
            if not isinstance(alloc, mybir.MemoryLocationSet):
                continue
            name = alloc.memorylocations[0].name
            if alloc.kind == 'ExternalInput':
                if name != partition_name:
                    in_names.append(name)
            elif alloc.kind == 'ExternalOutput':
                out_names.append(name)
                shape = tuple(alloc.tensor_shape)
                dtype = mybir.dt.np(alloc.dtype)
                out_avals.append(jax.core.ShapedArray(shape, dtype))
                zero_outs.append((shape, dtype))
        assert nc.dbg_addr is None
        n_params = len(in_names)
        in_names_all = in_names + out_names
        if partition_name is not None:
            in_names_all.append(partition_name)
        donate = tuple(range(n_params, n_params + len(out_avals)))
        self.in_names, self.out_names = in_names, out_names
        self.out_avals, self.zero_outs = out_avals, zero_outs

        def _body(*args):
            operands = list(args)
            if partition_name is not None:
                operands.append(partition_id_tensor())
            outs = _bass_exec_p.bind(
                *operands, out_avals=tuple(out_avals),
                in_names=tuple(in_names_all), out_names=tuple(out_names),
                lowering_input_output_aliases=(), sim_require_finite=True,
                sim_require_nnan=True, nc=nc)
            return tuple(outs)

        devices = jax.devices()[:ncores]
        assert len(devices) == ncores
        mesh = Mesh(np.asarray(devices), ('core',))
        nio = n_params + len(out_avals)
        self.sharded = jax.jit(
            shard_map(_body, mesh=mesh,
                      in_specs=(PartitionSpec('core'),) * nio,
                      out_specs=(PartitionSpec('core'),) * len(out_names),
                      check_rep=False),
            donate_argnums=donate, keep_unused=True)

    def __call__(self, concat_in):
        zeros = [np.zeros((self.ncores * s[0], *s[1:]), d)
                 for s, d in self.zero_outs]
        outs = self.sharded(*concat_in, *zeros)
        return {name: np.asarray(o) for name, o in zip(self.out_names, outs)}


def prepare_all(cfg, x, edge_index, W1, att_src1, att_dst1, b1,
                W2, att_src2, att_dst2, b2, W3, att_src3, att_dst3, b3):
    # self-loops (PyG add_self_loops) are handled analytically in the
    # normalize/final phases from the local h_own rows, so they are NOT
    # materialized as edges: smaller tables and less gather padding skew.
    src = np.asarray(edge_index[0]).astype(np.int64)
    dst = np.asarray(edge_index[1]).astype(np.int64)
    NP, per_core = _prepare_edges(cfg, src, dst)
    W1e_ = _ext_w(np.asarray(W1, np.float32), np.asarray(att_src1, np.float32),
                  np.asarray(att_dst1, np.float32))
    W2e_ = _ext_w(np.asarray(W2, np.float32), np.asarray(att_src2, np.float32),
                  np.asarray(att_dst2, np.float32))
    W3e_ = _ext_w(np.asarray(W3, np.float32), np.asarray(att_src3, np.float32),
                  np.asarray(att_dst3, np.float32))
    x = np.asarray(x, np.float32)
    wb_ = np.ascontiguousarray(np.concatenate([W1e_, W2e_, W3e_], axis=1))
    wblob = np.zeros(8 * cfg.WSLI, np.int16)
    wblob[0:cfg.WLEN] = wb_.view(np.int16).ravel()
    wblob[cfg.WLEN:cfg.WLEN + cfg.BLEN] = np.concatenate(
        [np.asarray(b1).ravel(), np.asarray(b2).ravel(),
         np.asarray(b3).ravel()]).astype(np.float32).view(np.int16)
    in_maps = []
    lvl = 63 if cfg.XBITS == 7 else 127
    for c in range(cfg.NCORES):
        xc = x[c * cfg.NPC:(c + 1) * cfg.NPC]
        sc = np.maximum(np.abs(xc).max(axis=1), 1e-30) / lvl     # [NPC]
        xq = np.clip(np.round(xc / sc[:, None]), -lvl, lvl).astype(np.int8)
        scp = np.ones(cfg.NPCP, np.float32)
        scp[:cfg.NPC] = sc
        xs_flat = scp.reshape(cfg.NT, 128).T.astype(np.float32).ravel()
        if cfg.XBITS == 7:
            up = np.full((cfg.NPCP, cfg.F), 64, np.uint8)   # pads: v=0
            up[:cfg.NPC] = (xq.astype(np.int16) + 64).astype(np.uint8)
            ut = np.ascontiguousarray(up.T).reshape(cfg.F, cfg.NPCP // 8, 8)
            bits = ((ut[:, :, 7:] >> np.arange(7)) & 1).astype(np.uint8)
            xpart = np.ascontiguousarray(
                ut[:, :, 0:7] | (bits << 7)).ravel().view(np.int16)
        else:
            xpart = np.ascontiguousarray(xq.T).ravel().view(np.int16)
        wchunk = (wblob[c * cfg.WSLI:(c + 1) * cfg.WSLI] if cfg.WAG
                  else wblob)
        blob = np.concatenate([
            per_core[c]['ei'].ravel(),
            xpart,
            wchunk,
            xs_flat.view(np.int16)])
        in_maps.append(np.ascontiguousarray(blob)[None, :])
    return NP, in_maps


_CACHE = {}
LAST_RESULT = None
LAST_RUN = None


def run_again():
    import time
    runner, concat_in, cfg = LAST_RUN
    t0 = time.monotonic()
    runner(concat_in)
    return time.monotonic() - t0


def kernel(x, edge_index, W1, att_src1, att_dst1, b1, W2, att_src2, att_dst2,
           b2, W3, att_src3, att_dst3, b3):
    global LAST_RESULT, LAST_RUN
    x = np.asarray(x)
    edge_index = np.asarray(edge_index)
    cfg = Cfg(N=x.shape[0], F=x.shape[1], H=np.asarray(att_src1).shape[0],
              C=np.asarray(att_src1).shape[1], OUT=np.asarray(W3).shape[1])
    NP, in_maps = prepare_all(cfg, x, edge_index, W1, att_src1, att_dst1,
                              b1, W2, att_src2, att_dst2, b2, W3, att_src3,
                              att_dst3, b3)
    key = (cfg.N, cfg.F, NP)
    if key not in _CACHE:
        nc = _build(cfg)
        _CACHE[key] = _Runner(nc, cfg.NCORES)
    runner = _CACHE[key]
    concat_in = [np.concatenate(in_maps, axis=0)]
    LAST_RUN = (runner, concat_in, cfg)
    res = runner(concat_in)
    LAST_RESULT = res
    oo = res['out_own'].reshape(cfg.NCORES, cfg.NPCP, cfg.OUT)
    out = np.concatenate([oo[c][:cfg.NPC] for c in range(cfg.NCORES)], axis=0)
    return out.astype(np.float32)


# revision 49
# speedup vs baseline: 1.0315x; 1.0315x over previous
"""3-layer GAT on 8 Trainium2 NeuronCores (Bass/Tile) — v7.

Same math as the v2 baseline (1D node-parallel, deferred-softmax GAT).
The measured warm-call wall under axon is transfer-dominated (~73ms RPC
floor + ~11ms/MB on the tunneled link + ~19ms device exec), so v3-v7
attack the host->device path as much as the device program:

  - Persistent jit runner: the shard_map'd bass_exec executable is built
    once and reused across calls (v2 re-jit'd per call via
    run_bass_kernel_spmd, re-loading the NEFF each time: ~50ms/call).
  - Wire diet (19.7MB -> 14.7MB): the dstw8 edge table is dropped (the
    int16 dst gather index is derived on device from dstf8 via a
    wrap-conversion shuffle done with 16-partition DMAs + max(.,0));
    the weight/bias blob is uploaded sharded 1/8-per-core and
    AllGather'd on device instead of 8x-replicated host-side; x is
    quantized to 7 bits (8 values packed in 7 bytes, MSB-folded,
    per-row scale max/63; unpacked on DVE with bitwise_and + is_lt).
    Final max rel err ~1.1e-2 vs the 2e-2 gate.
  - Self-loops (PyG add_self_loops) are computed analytically in the
    normalize/final phases from local h_own rows instead of being
    materialized as edges: smaller tables and NP 1408 -> 1280.
  - Edges are sorted by src within each dst window (order is free):
    ascending HBM rows for the gather, compressible idx table.
  - UNROLL=2 on the hw loops: with a persistent executable, program size
    is a one-time cost, so double-buffered loop bodies are affordable.

Per layer: transform (h|s|d = z @ Wext, node tiles via hw loop) -> AllGather
of the padded node table -> edge phase (hw loop over 49 dst windows: gather
src rows + dst coefficients, ee = exp(leaky_relu(s+d)), messages = h*ee,
one-hot matmul aggregation into PSUM, PSUM -> acc DRAM) -> normalize
(self-loop term + out = num/den + b, ELU, transpose back into zT).

Known-even alternative: replacing the per-edge dst-coefficient gather
with sel^T (DMA-transpose) x d_win matmuls measured identical wall time;
kept the simpler gather form.
"""

import sys

import numpy as np

sys.path.insert(0, '/opt/trn_rl_repo')

from contextlib import ExitStack

import ml_dtypes

import jax  # noqa: E402

try:
    jax.config.update('jax_compilation_cache_dir', '/tmp/jax_comp_cache')
    jax.config.update('jax_persistent_cache_min_compile_time_secs', 0)
    jax.config.update('jax_persistent_cache_min_entry_size_bytes', 0)
except Exception:
    pass

from jax.sharding import Mesh, PartitionSpec  # noqa: E402
from jax.experimental.shard_map import shard_map  # noqa: E402

from concourse import bacc, mybir, tile  # noqa: E402
from concourse import bass2jax  # noqa: E402
from concourse.bass import ds, ts  # noqa: E402
from concourse.bass2jax import _bass_exec_p, partition_id_tensor  # noqa: E402
from concourse.masks import make_identity  # noqa: E402

F32 = mybir.dt.float32
BF16 = mybir.dt.bfloat16
I16 = mybir.dt.int16
AF = mybir.ActivationFunctionType
ALU = mybir.AluOpType
BF = ml_dtypes.bfloat16


class Cfg:
    def __init__(self, N=50000, F=256, H=8, C=32, OUT=2, NCORES=8, NP=1280,
                 GS=768, UNROLL=2, HWLOOP=True, WAG=True, XBITS=7):
        self.N, self.F, self.H, self.C, self.OUT = N, F, H, C, OUT
        self.NCORES = NCORES
        assert N % NCORES == 0 and F % 128 == 0
        self.KH = F // 128
        self.NPC = N // NCORES                 # real nodes per core
        self.NT = (self.NPC + 127) // 128      # node tiles / edge windows
        self.NPCP = self.NT * 128              # padded nodes per core
        self.NG = NCORES * self.NPCP           # padded global nodes
        self.SPLIT = (NCORES // 2) * self.NPCP  # stream boundary (int16 idx)
        assert self.SPLIT <= 32768 and self.NG - self.SPLIT <= 32768
        self.EW = 384                          # l1-2 row elems (768B, bf16)
        self.EW3 = 128                         # l3 row elems (256B)
        self.WC = F + 2 * H                    # transform cols (h|s|d)
        self.WC3 = OUT + 2
        self.MC = F + H                        # aggregated cols (msg|ee)
        self.MC3 = OUT + 1
        self.NP = NP                           # padded edges per (stream,win)
        assert NP % 128 == 0
        self.GS = GS                           # idxs per dma_gather call
        assert GS % 128 == 0
        self.UNROLL = UNROLL
        self.HWLOOP = HWLOOP
        self.WAG = WAG
        # weight blob: W1e|W2e|W3e (bf16) + b1|b2|b3 (f32), sharded 1/8
        self.WLEN = F * (2 * self.WC + self.WC3)      # i16 units
        self.BLEN = 2 * (2 * F + OUT)                 # i16 units (f32 biases)
        self.WSLI = ((self.WLEN + self.BLEN + 2 * F + 8 * 256 - 1)
                     // (8 * 256)) * 256              # per-core slice, i16
        self.WREG = self.WSLI if WAG else 8 * self.WSLI  # ei region size
        # x quantization: 8 = plain int8; 7 = 7-bit (8 vals packed in 7
        # bytes, MSB-folded; per-row scale max/63). 7-bit keeps the final
        # max rel err ~1.2e-2 (<2e-2 gate) and cuts 1.56MB off the wire.
        self.XBITS = XBITS
        assert XBITS in (7, 8) and self.NPCP % 8 == 0
        if XBITS == 7:
            self.XLEN = F * (self.NPCP // 8) * 7 // 2    # i16 units
        else:
            self.XLEN = F * self.NPC // 2


def _gather_chunks(cfg):
    """[(idx_off, n_idx), ...] covering NP in <=GS multiples of 128."""
    out, off = [], 0
    while off < cfg.NP:
        n = min(cfg.GS, cfg.NP - off)
        out.append((off, n))
        off += n
    return out


def _amat(att):
    Hh, Cc = att.shape
    A = np.zeros((Hh * Cc, Hh), np.float32)
    for h in range(Hh):
        A[h * Cc:(h + 1) * Cc, h] = att[h]
    return A


def _ext_w(W, a_s, a_d):
    Ws = (W @ _amat(a_s)).astype(np.float32)
    Wd = (W @ _amat(a_d)).astype(np.float32)
    return np.ascontiguousarray(
        np.concatenate([W, Ws, Wd], axis=1)).astype(BF)


def _wrap16(arr):
    return np.ascontiguousarray(arr.reshape(-1, 16).T.astype(np.int16))


def _prepare_edges(cfg, src, dst):
    """Host-side partitioning. Returns (NP, per_core list of dicts).

    Per (core, stream): src idx table (int16, 16-wrapped) and dstf table
    (int8, 128-wrapped; window-local dst row 0..127, -1 for pad slots).
    The device derives the int16 dst gather index as max(dstf, 0) via a
    wrap-conversion shuffle, so pads gather (finite) row 0 while their
    one-hot column stays all-zero.
    """
    NC, NPC, NPCP, NT = cfg.NCORES, cfg.NPC, cfg.NPCP, cfg.NT
    core_of = dst // NPC
    dstl = dst - core_of * NPC             # 0..NPC-1 (== padded local row)
    win = dstl // 128
    dstw = dstl - win * 128
    srcp = (src // NPC) * NPCP + (src % NPC)   # padded global row
    stream = (srcp >= cfg.SPLIT).astype(np.int64)

    counts = np.zeros((NC, 2, NT), np.int64)
    np.add.at(counts, (core_of, stream, win), 1)
    NP = int(((max(counts.max(), 1) + 127) // 128) * 128)
    cfg.NP = NP
    T = NT * NP

    per_core = []
    for c in range(NC):
        out = {}
        for s, tag in ((0, 'A'), (1, 'B')):
            src_a = np.zeros(T, np.int64)
            dstf_a = -np.ones(T, np.float32)
            m = (core_of == c) & (stream == s)
            ww = win[m]
            sp = srcp[m]
            # secondary sort by src: edge order within a window is free
            # (one-hot aggregation commutes), and sorted src makes the
            # int16 table compressible for the wire and the hg gather
            # walk HBM rows in ascending order.
            order = np.lexsort((sp, ww))
            ww = ww[order]
            ss = sp[order] - s * cfg.SPLIT
            fw = dstw[m][order]
            grp_start = np.searchsorted(ww, np.arange(NT))
            rank = np.arange(len(ww)) - grp_start[ww]
            pos = ww * NP + rank
            src_a[pos] = ss
            dstf_a[pos] = fw
            out['src' + tag] = _wrap16(src_a)
            out['dstf8' + tag] = np.ascontiguousarray(
                dstf_a.reshape(-1, 128).T.astype(np.int8))
        blob = np.concatenate([
            out['srcA'].ravel(), out['srcB'].ravel(),
            np.concatenate([out['dstf8A'].ravel(),
                            out['dstf8B'].ravel()]).view(np.int16)])
        per_core.append({'ei': np.ascontiguousarray(blob)[None, :]})
    return NP, per_core


def _build(cfg, abl=''):
    # abl: debug-only ablation for timing probes ('' = full kernel):
    #   'noedge' - skip edge phases; 'nodg' - skip dst-coeff gathers;
    #   'gonly'  - gathers without edge compute; 'noag' - no collectives.
    NC, NT, NP, KH = cfg.NCORES, cfg.NT, cfg.NP, cfg.KH
    F, H, C, OUT = cfg.F, cfg.H, cfg.C, cfg.OUT
    NPC, NPCP, NG = cfg.NPC, cfg.NPCP, cfg.NG
    EW, EW3, WC, WC3, MC, MC3 = (cfg.EW, cfg.EW3, cfg.WC, cfg.WC3, cfg.MC,
                                 cfg.MC3)
    T16 = NT * NP // 16
    T128 = NT * NP // 128
    NS = NP // 128                      # 128-edge slices per (stream, window)
    chunks = _gather_chunks(cfg)

    nc = bacc.Bacc('TRN2', target_bir_lowering=False, debug=False,
                   num_devices=NC)

    # ---- I/O ----
    T = NT * NP
    # ALL inputs are packed into ONE int16 tensor (bitcast views carve out
    # int8/bf16/f32 regions): PJRT-over-axon charges a fixed cost per input
    # array per call, so array count matters as much as bytes.
    # Layout (i16 units): srcA|srcB (16-wrapped int16), dstf8A|dstf8B
    # (128-wrapped int8 window offsets, -1 pads), xT (int8 pairs),
    # wsl (per-core 1/8 slice of W1e|W2e|W3e bf16 + b1|b2|b3 f32),
    # fb (f32 x row scales).
    XOFF = 3 * T
    XLEN = cfg.XLEN
    WOFF = XOFF + XLEN
    WSLI = cfg.WSLI
    FOFF = WOFF + cfg.WREG
    NF32 = 128 * NT
    assert XOFF % 2 == 0 and FOFF % 2 == 0
    ei = nc.dram_tensor('ei', [1, FOFF + 2 * NF32], I16,
                        kind='ExternalInput')
    xT = ei[0:1, XOFF:XOFF + XLEN].bitcast(mybir.dt.int8).rearrange(
        'x (p c) -> (x p) c', p=F)
    fb = ei[0:1, FOFF:FOFF + 2 * NF32].bitcast(F32)
    out_own = nc.dram_tensor('out_own', [NPCP, OUT], mybir.dt.float16,
                             kind='ExternalOutput')

    # ---- internal DRAM ----
    h_own = nc.dram_tensor('h_own', [NPCP, EW], BF16)
    hc_own = nc.dram_tensor('hc_own', [NPCP, F], BF16)
    hc_full = nc.dram_tensor('hc_full', [NG, F], BF16, addr_space='Shared')
    h3_own = nc.dram_tensor('h3_own', [NPCP, EW3], BF16)
    h3_full = nc.dram_tensor('h3_full', [NG, EW3], BF16, addr_space='Shared')
    acc_d = nc.dram_tensor('acc', [NPCP, MC], F32)
    acc3_d = nc.dram_tensor('acc3', [NPCP, MC3], F32)
    if cfg.WAG:
        wsl_d = nc.dram_tensor('wsl', [128, WSLI // 128], I16)
        wfull_d = nc.dram_tensor('wfull', [NC * 128, WSLI // 128], I16,
                                 addr_space='Shared')

    with tile.TileContext(nc) as tc, ExitStack() as ctx:
        const = ctx.enter_context(tc.tile_pool(name='const', bufs=1))
        sb = ctx.enter_context(tc.tile_pool(name='sb', bufs=2))
        eb = ctx.enter_context(tc.tile_pool(name='eb', bufs=2))
        ps = ctx.enter_context(tc.tile_pool(name='ps', bufs=2, space='PSUM'))
        ps1 = ctx.enter_context(tc.tile_pool(name='ps1', bufs=1,
                                             space='PSUM'))

        # ---- weight allgather: 1/8 slice -> full blob on every core ----
        if cfg.WAG:
            with tc.tile_pool(name='wstg', bufs=1) as wstg:
                wt_s = wstg.tile([128, WSLI // 128], I16, tag='wsl')
                nc.sync.dma_start(
                    out=wt_s[:],
                    in_=ei[0:1, WOFF:WOFF + WSLI].rearrange(
                        'x (p c) -> (x p) c', p=128))
                nc.sync.dma_start(out=wsl_d[:, :], in_=wt_s[:])
            nc.gpsimd.collective_compute(
                'AllGather', ALU.bypass, ins=[wsl_d[:, :]],
                outs=[wfull_d[:, :]], replica_groups=[list(range(NC))])
            wflat = wfull_d[:, :].rearrange('(x p) c -> x (p c)', x=1)
        else:
            wflat = ei[0:1, WOFF:WOFF + 8 * WSLI]
        wb = wflat[0:1, 0:cfg.WLEN].bitcast(BF16).rearrange(
            'x (p c) -> (x p) c', p=F)
        fbw = wflat[0:1, cfg.WLEN:cfg.WLEN + cfg.BLEN].bitcast(F32)

        # ---- constants / resident tables ----
        iota_t = const.tile([128, 128], BF16)
        nc.gpsimd.iota(iota_t[:], pattern=[[1, 128]], base=0,
                       channel_multiplier=0,
                       allow_small_or_imprecise_dtypes=True)
        ident = const.tile([128, 128], F32)
        make_identity(nc, ident[:])
        b_t = {}
        for name, off, w in (('b1', 0, F), ('b2', F, F), ('b3', 2 * F, OUT)):
            t = const.tile([128, w], F32, tag='b_' + name)
            nc.sync.dma_start(out=t[:],
                              in_=fbw[:, off:off + w].to_broadcast((128, w)))
            b_t[name] = t
        a_t = {}
        for name, l in (('a1', 0), ('a2', 1)):
            t = const.tile([128, F], BF16, tag='a_' + name)
            aoff = cfg.WLEN + cfg.BLEN + l * F
            nc.sync.dma_start(
                out=t[:],
                in_=wflat[0:1, aoff:aoff + F].bitcast(BF16).to_broadcast(
                    (128, F)))
            a_t[name] = t
        w_t = {}
        for name, off, w in (('W1', 0, WC), ('W2', WC, WC),
                             ('W3', 2 * WC, WC3)):
            t = const.tile([128, KH, w], BF16, tag='w_' + name)
            for kh in range(KH):
                nc.sync.dma_start(out=t[:, kh, :],
                                  in_=wb[kh * 128:(kh + 1) * 128,
                                         off:off + w])
            w_t[name] = t
        idx_t = {}
        for bi, tag in ((0, 'A'), (1, 'B')):
            t = const.tile([128, T16], I16, tag='src' + tag)
            view = ei[0:1, bi * T:(bi + 1) * T].rearrange(
                'x (p c) -> (x p) c', p=16)
            for k in range(8):
                nc.sync.dma_start(out=t[16 * k:16 * (k + 1), :], in_=view)
            idx_t['src' + tag] = t
        ei8 = ei[0:1, 2 * T:3 * T].bitcast(mybir.dt.int8)   # [1, 2T] bytes
        with tc.tile_pool(name='stg', bufs=1) as stg:
            st = stg.tile([128, T128], mybir.dt.int8, tag='stg8')
            for si, tag in ((0, 'A'), (1, 'B')):
                nc.sync.dma_start(
                    out=st[:, 0:T128],
                    in_=ei8[0:1, si * T:(si + 1) * T].rearrange(
                        'x (p c) -> (x p) c', p=128))
                tf = const.tile([128, T128], BF16, tag='dstf' + tag)
                nc.vector.tensor_copy(out=tf[:], in_=st[:, 0:T128])
                idx_t['dstf' + tag] = tf
                # derive the 16-wrapped int16 dst gather index from the
                # 128-wrapped bytes: idx16[16a+r, c] = dstf[(c%8)*16+r, c//8].
                # Convert+clamp full-width on DVE (partition starts must be
                # 0/32/64/96), then shuffle with 16-partition SBUF DMAs.
                sti = stg.tile([128, T128], I16, tag='stg16')
                nc.vector.tensor_copy(out=sti[:], in_=st[:, 0:T128])
                nc.vector.tensor_scalar_max(sti[:], sti[:], 0)
                ti = const.tile([128, T16], I16, tag='dst' + tag)
                tiv = ti.rearrange('p (c m) -> p c m', m=8)
                for a in range(8):
                    for m in range(8):
                        nc.sync.dma_start(
                            out=tiv[16 * a:16 * (a + 1), :, m],
                            in_=sti[16 * m:16 * (m + 1), :])
                idx_t['dst' + tag] = ti
        zT = const.tile([128, KH, NPCP], BF16)
        xs_t = const.tile([128, NT], F32)
        nc.sync.dma_start(out=xs_t[:], in_=fb[:, 0:NF32].rearrange(
            'x (p t) -> (x p) t', p=128))

        def loop(n, body, unroll):
            if cfg.HWLOOP and n > unroll:
                tc.For_i_unrolled(0, n, 1, body, max_unroll=unroll)
            else:
                for i in range(n):
                    body(i)

        # ---- phase bodies ----
        def transform_body(i, wt, wc, hout, ew, scaled=False, hcout=None):
            pool = ps if wc > 16 else ps1
            p = pool.tile([128, wc], F32, tag=f'ps_tr{wc}')
            # walrus can't take register offsets on the ldweights operand, so
            # stage the dynamic zT slice into a static tile first.
            zs = sb.tile([128, KH, 128], BF16, tag='sb_zs')
            nc.vector.tensor_copy(out=zs[:], in_=zT[:, :, ts(i, 128)])
            for kh in range(KH):
                nc.tensor.matmul(p[:, :], lhsT=zs[:, kh, :],
                                 rhs=wt[:, kh, :], start=(kh == 0),
                                 stop=(kh == KH - 1))
            ht = sb.tile([128, wc], BF16, tag=f'sb_tr{wc}')
            if scaled:
                # undo the int8 row quantization of x (scale is per node,
                # nodes sit on partitions here)
                nc.vector.tensor_tensor(
                    out=ht[:], in0=p[:],
                    in1=xs_t[:, ts(i, 1)].to_broadcast((128, wc)),
                    op=ALU.mult)
            else:
                nc.vector.tensor_copy(out=ht[:], in_=p[:])
            nc.sync.dma_start(out=hout[ts(i, 128), 0:wc], in_=ht[:])
            if hcout is not None:
                nc.sync.dma_start(out=hcout[ts(i, 128), :], in_=ht[:, 0:F])

        def edge_body(w, tblf, dtbl, dcol, ew, mc, hh, cc, s_off, d_off,
                      acc, abc=None):
            """One destination window: gather, ee, messages, aggregate."""
            pool = ps if mc > 16 else ps1
            mm = []  # (lhsT slice, rhs slice) accumulation chain
            for si, (s, tag) in enumerate(((0, 'A'), (1, 'B'))):
                tbl = (tblf[0:cfg.SPLIT, :] if s == 0 else
                       tblf[cfg.SPLIT:NG, :])
                hg = eb.tile([128, NS, ew], BF16, tag=f'hg{ew}{tag}')
                dg = eb.tile([128, NS, 128], BF16, tag=f'dg{tag}')
                for off, ni in chunks:
                    i16 = w * (NP // 16) + off // 16
                    sl = off // 128
                    nc.gpsimd.dma_gather(
                        hg[:, sl:sl + ni // 128, :], tbl,
                        idx_t['src' + tag][:, ds(i16, ni // 16)],
                        num_idxs=ni, num_idxs_reg=ni, elem_size=ew)
                    if abl != 'nodg':
                        nc.gpsimd.dma_gather(
                            dg[:, sl:sl + ni // 128, :],
                            dtbl[ds(w * 128, 128), dcol:dcol + 128],
                            idx_t['dst' + tag][:, ds(i16, ni // 16)],
                            num_idxs=ni, num_idxs_reg=ni, elem_size=128,
                            elem_step=(EW if abc is not None else ew))
                if abl == 'nodg':
                    nc.vector.memset(dg[:], 0)
                if abl == 'gonly':
                    gc = sb.tile([128, 1], F32, tag=f'gonly_c{tag}')
                    nc.vector.tensor_tensor(out=gc[:], in0=hg[:, 0, 0:1],
                                            in1=dg[:, 0, 0:1], op=ALU.add)
                    nc.sync.dma_start(
                        out=acc[ts(w, 128), si:si + 1], in_=gc[:])
                    continue
                e8_t = eb.tile([128, NS, H], F32, tag=f'e8{tag}')
                e8 = e8_t[:, :, 0:hh]
                msg_t = eb.tile([128, NS, MC], BF16, tag=f'msg{tag}')
                msg = msg_t[:, :, 0:mc]
                if abc is not None:
                    # gathered rows are compact (h only, 512B): compute the
                    # src coefficient on-device, s = sum_c h[h,c]*a_src[h,c],
                    # using the msg tile as scratch for the product.
                    nc.vector.tensor_tensor(
                        out=msg[:, :, 0:F], in0=hg[:],
                        in1=abc[:, None, :].to_broadcast((128, NS, F)),
                        op=ALU.mult)
                    nc.vector.tensor_reduce(
                        out=e8[:],
                        in_=msg[:, :, 0:F].rearrange(
                            'p n (h c) -> p n h c', h=hh),
                        axis=mybir.AxisListType.X, op=ALU.add)
                    nc.vector.tensor_tensor(
                        out=e8[:], in0=e8[:],
                        in1=dg[:, :, d_off:d_off + hh], op=ALU.add)
                else:
                    nc.vector.tensor_tensor(
                        out=e8[:], in0=hg[:, :, s_off:s_off + hh],
                        in1=dg[:, :, d_off:d_off + hh], op=ALU.add)
                el_t = eb.tile([128, NS, H], F32, tag=f'el{tag}')
                el = el_t[:, :, 0:hh]
                nc.vector.tensor_scalar_mul(el[:], e8[:], 0.2)
                nc.vector.tensor_tensor(out=el[:], in0=el[:], in1=e8[:],
                                        op=ALU.max)
                nc.scalar.activation(msg[:, :, hh * cc:hh * cc + hh], el[:],
                                     AF.Exp)
                nc.vector.tensor_tensor(
                    out=msg[:, :, 0:hh * cc].rearrange(
                        'p n (h c) -> p n h c', h=hh),
                    in0=hg[:, :, 0:hh * cc].rearrange(
                        'p n (h c) -> p n h c', h=hh),
                    in1=msg[:, :, hh * cc:hh * cc + hh][
                        :, :, :, None].to_broadcast((128, NS, hh, cc)),
                    op=ALU.mult)
                sel = eb.tile([128, NS, 128], BF16, tag=f'sel{tag}')
                nc.vector.tensor_tensor(
                    out=sel[:],
                    in0=iota_t[:, None, :].to_broadcast((128, NS, 128)),
                    in1=idx_t['dstf' + tag][:, ts(w, NS), None].to_broadcast(
                        (128, NS, 128)),
                    op=ALU.is_equal)
                for j in range(NS):
                    mm.append((sel[:, j, :], msg[:, j, :]))
            if abl == 'gonly':
                return
            p = pool.tile([128, mc], F32, tag=f'ps_agg{mc}')
            for j, (lh, rh) in enumerate(mm):
                nc.tensor.matmul(p[:, :], lhsT=lh, rhs=rh, start=(j == 0),
                                 stop=(j == len(mm) - 1))
            ac = sb.tile([128, mc], F32, tag=f'ac{mc}')
            nc.vector.tensor_copy(out=ac[:], in_=p[:])
            nc.sync.dma_start(out=acc[ts(w, 128), :], in_=ac[:])

        def normalize_body(i, bt):
            a = sb.tile([128, MC], F32, tag='nrm_a')
            nc.sync.dma_start(out=a[:], in_=acc_d[ts(i, 128), :])
            # self-loop contribution: ee = exp(lrelu(s_i + d_i)),
            # num += h_i * ee, den += ee (rows are local, no gather needed)
            hh = sb.tile([128, WC], BF16, tag='nrm_hh')
            nc.sync.dma_start(out=hh[:], in_=h_own[ts(i, 128), 0:WC])
            tsd = sb.tile([128, H], F32, tag='nrm_t')
            nc.vector.tensor_tensor(out=tsd[:], in0=hh[:, F:F + H],
                                    in1=hh[:, F + H:F + 2 * H], op=ALU.add)
            tl = sb.tile([128, H], F32, tag='nrm_tl')
            nc.vector.tensor_scalar_mul(tl[:], tsd[:], 0.2)
            nc.vector.tensor_tensor(out=tl[:], in0=tl[:], in1=tsd[:],
                                    op=ALU.max)
            es = sb.tile([128, H], F32, tag='nrm_es')
            nc.scalar.activation(es[:], tl[:], AF.Exp)
            nc.vector.tensor_tensor(out=a[:, F:F + H], in0=a[:, F:F + H],
                                    in1=es[:], op=ALU.add)
            hm = sb.tile([128, F], F32, tag='nrm_hm')
            nc.vector.tensor_tensor(
                out=hm[:].rearrange('p (h c) -> p h c', h=H),
                in0=hh[:, 0:F].rearrange('p (h c) -> p h c', h=H),
                in1=es[:, :, None].to_broadcast((128, H, C)), op=ALU.mult)
            nc.vector.tensor_tensor(out=a[:, 0:F], in0=a[:, 0:F], in1=hm[:],
                                    op=ALU.add)
            r = sb.tile([128, H], F32, tag='nrm_r')
            nc.vector.tensor_scalar_add(r[:], a[:, F:F + H], 1e-16)
            rr = sb.tile([128, H], F32, tag='nrm_rr')
            nc.vector.reciprocal(rr[:], r[:])
            z = sb.tile([128, F], F32, tag='nrm_z')
            nc.vector.tensor_tensor(
                out=z[:].rearrange('p (h c) -> p h c', h=H),
                in0=a[:, 0:F].rearrange('p (h c) -> p h c', h=H),
                in1=rr[:, :, None].to_broadcast((128, H, C)), op=ALU.mult)
            nc.vector.tensor_tensor(out=z[:], in0=z[:], in1=bt[:],
                                    op=ALU.add)
            # ELU: max(z,0) + exp(min(z,0)) - 1
            zn = sb.tile([128, F], F32, tag='nrm_zn')
            nc.vector.tensor_scalar_min(zn[:], z[:], 0.0)
            en = sb.tile([128, F], F32, tag='nrm_en')
            nc.scalar.activation(en[:], zn[:], AF.Exp)
            nc.vector.tensor_scalar_add(en[:], en[:], -1.0)
            nc.vector.tensor_scalar_max(z[:], z[:], 0.0)
            nc.vector.tensor_tensor(out=z[:], in0=z[:], in1=en[:], op=ALU.add)
            for kh in range(KH):
                tp = ps1.tile([128, 128], F32, tag='ps_tp')
                nc.tensor.transpose(out=tp[:, :],
                                    in_=z[:, kh * 128:(kh + 1) * 128],
                                    identity=ident[:, :])
                nc.vector.tensor_copy(out=zT[:, kh, ts(i, 128)], in_=tp[:, :])

        def final_body(i):
            a = sb.tile([128, MC3], F32, tag='fo_a')
            nc.sync.dma_start(out=a[:], in_=acc3_d[ts(i, 128), :])
            hh = sb.tile([128, WC3], BF16, tag='fo_hh')
            nc.sync.dma_start(out=hh[:], in_=h3_own[ts(i, 128), 0:WC3])
            tsd = sb.tile([128, 1], F32, tag='fo_t')
            nc.vector.tensor_tensor(out=tsd[:], in0=hh[:, OUT:OUT + 1],
                                    in1=hh[:, OUT + 1:OUT + 2], op=ALU.add)
            tl = sb.tile([128, 1], F32, tag='fo_tl')
            nc.vector.tensor_scalar_mul(tl[:], tsd[:], 0.2)
            nc.vector.tensor_tensor(out=tl[:], in0=tl[:], in1=tsd[:],
                                    op=ALU.max)
            es = sb.tile([128, 1], F32, tag='fo_es')
            nc.scalar.activation(es[:], tl[:], AF.Exp)
            nc.vector.tensor_tensor(out=a[:, OUT:OUT + 1],
                                    in0=a[:, OUT:OUT + 1], in1=es[:],
                                    op=ALU.add)
            hm = sb.tile([128, OUT], F32, tag='fo_hm')
            nc.vector.tensor_tensor(out=hm[:], in0=hh[:, 0:OUT],
                                    in1=es[:, :].to_broadcast((128, OUT)),
                                    op=ALU.mult)
            nc.vector.tensor_tensor(out=a[:, 0:OUT], in0=a[:, 0:OUT],
                                    in1=hm[:], op=ALU.add)
            r = sb.tile([128, 1], F32, tag='fo_r')
            nc.vector.tensor_scalar_add(r[:], a[:, OUT:OUT + 1], 1e-16)
            rr = sb.tile([128, 1], F32, tag='fo_rr')
            nc.vector.reciprocal(rr[:], r[:])
            o = sb.tile([128, OUT], F32, tag='fo_o')
            nc.vector.tensor_tensor(out=o[:], in0=a[:, 0:OUT],
                                    in1=rr[:, :].to_broadcast((128, OUT)),
                                    op=ALU.mult)
            o16 = sb.tile([128, OUT], mybir.dt.float16, tag='fo_o16')
            nc.vector.tensor_tensor(out=o16[:], in0=o[:], in1=b_t['b3'][:],
                                    op=ALU.add)
            nc.sync.dma_start(out=out_own[ts(i, 128), :], in_=o16[:])

        def allgather(src_d, dst_d):
            nc.gpsimd.collective_compute(
                'AllGather', ALU.bypass, ins=[src_d[:, :]], outs=[dst_d[:, :]],
                replica_groups=[list(range(NC))])

        # ---------------- program ----------------
        U = cfg.UNROLL
        with nc.named_scope('tr1'):
            # Zero-fill node tables once: the pad columns (WC:EW) are never
            # consumed, but stale DRAM would trip the sim's finite check.
            zpad = const.tile([128, EW], BF16)
            nc.vector.memset(zpad[:], 0)
            nc.sync.dma_start(
                out=h_own[:, :].rearrange('(t p) e -> p t e', p=128),
                in_=zpad[:, None, :].to_broadcast((128, NT, EW)))
            nc.sync.dma_start(
                out=h3_own[:, :].rearrange('(t p) e -> p t e', p=128),
                in_=zpad[:, None, 0:EW3].to_broadcast((128, NT, EW3)))
            if abl == 'noedge':
                zp32 = const.tile([128, MC], F32, tag='ablz')
                nc.vector.memset(zp32[:], 0)
                nc.sync.dma_start(
                    out=acc_d[:, :].rearrange('(t p) e -> p t e', p=128),
                    in_=zp32[:, None, 0:MC].to_broadcast((128, NT, MC)))
                nc.sync.dma_start(
                    out=acc3_d[:, :].rearrange('(t p) e -> p t e', p=128),
                    in_=zp32[:, None, 0:MC3].to_broadcast((128, NT, MC3)))
            if abl == 'noag':
                nc.sync.dma_start(
                    out=hc_full[:, :].rearrange('(t p) e -> p t e', p=128),
                    in_=zpad[:, None, 0:F].to_broadcast((128, NT * NC, F)))
                nc.sync.dma_start(
                    out=h3_full[:, :].rearrange('(t p) e -> p t e', p=128),
                    in_=zpad[:, None, 0:EW3].to_broadcast(
                        (128, NT * NC, EW3)))
            if cfg.XBITS == 7:
                # unpack: 8 values per 7 bytes; b_i holds u_i (low 7 bits)
                # and bit i of u_7 (MSB). v = u - 64.
                XB = NPCP // 8 * 7
                xp = const.tile([128, KH, XB], mybir.dt.int8)
                for kh in range(KH):
                    nc.sync.dma_start(out=xp[:, kh, :],
                                      in_=xT[kh * 128:(kh + 1) * 128, :])
                xu = const.tile([128, KH, NPCP], mybir.dt.int8)
                xpv = xp.rearrange('p k (g i) -> p k g i', i=7)
                xuv = xu.rearrange('p k (g i) -> p k g i', i=8)
                nc.vector.tensor_scalar(
                    out=xuv[:, :, :, 0:7], in0=xpv[:], scalar1=127,
                    scalar2=None, op0=ALU.bitwise_and)
                a7 = const.tile([128, KH, NPCP // 8], mybir.dt.int8,
                                tag='x7a')
                t7 = const.tile([128, KH, NPCP // 8], mybir.dt.int8,
                                tag='x7t')
                for i in range(7):
                    dstt = a7 if i == 0 else t7
                    nc.vector.tensor_scalar(
                        out=dstt[:], in0=xpv[:, :, :, i], scalar1=0,
                        scalar2=1 << i, op0=ALU.is_lt, op1=ALU.mult)
                    if i:
                        nc.vector.tensor_tensor(out=a7[:], in0=a7[:],
                                                in1=t7[:], op=ALU.add)
                nc.vector.tensor_copy(out=xuv[:, :, :, 7], in_=a7[:])
                nc.vector.tensor_scalar_add(zT[:, :, 0:NPCP], xu[:], -64.0)
            else:
                xi8 = const.tile([128, KH, NPC], mybir.dt.int8)
                for kh in range(KH):
                    nc.sync.dma_start(out=xi8[:, kh, :],
                                      in_=xT[kh * 128:(kh + 1) * 128, :])
                nc.vector.tensor_copy(out=zT[:, :, 0:NPC], in_=xi8[:])
                if NPCP > NPC:
                    nc.vector.memset(zT[:, :, NPC:NPCP], 0)
            loop(NT, lambda i: transform_body(i, w_t['W1'], WC, h_own, EW,
                                              scaled=True, hcout=hc_own), U)
        with nc.named_scope('ag1'):
            if abl != 'noag':
                allgather(hc_own, hc_full)
        with nc.named_scope('edges1'):
            if abl != 'noedge':
                loop(NT, lambda w: edge_body(
                    w, hc_full, h_own, F, F, MC, H, C, F, H, acc_d,
                    abc=a_t['a1']), U)
        with nc.named_scope('tr2'):
            loop(NT, lambda i: normalize_body(i, b_t['b1']), U)
            loop(NT, lambda i: transform_body(i, w_t['W2'], WC, h_own, EW,
                                              hcout=hc_own), U)
        with nc.named_scope('ag2'):
            if abl != 'noag':
                allgather(hc_own, hc_full)
        with nc.named_scope('edges2'):
            if abl != 'noedge':
                loop(NT, lambda w: edge_body(
                    w, hc_full, h_own, F, F, MC, H, C, F, H, acc_d,
                    abc=a_t['a2']), U)
        with nc.named_scope('tr3'):
            loop(NT, lambda i: normalize_body(i, b_t['b2']), U)
            loop(NT, lambda i: transform_body(i, w_t['W3'], WC3, h3_own, EW3),
                 U)
        with nc.named_scope('ag3'):
            if abl != 'noag':
                allgather(h3_own, h3_full)
        with nc.named_scope('edges3'):
            if abl != 'noedge':
                loop(NT, lambda w: edge_body(
                    w, h3_full, h3_own, 0, EW3, MC3, 1, OUT, OUT, OUT + 1,
                    acc3_d), U)
        with nc.named_scope('fin'):
            loop(NT, final_body, U)

    nc.compile()
    # The module is immutable from here on; memoize its serialization so the
    # per-call jax lowering doesn't redo ~20ms of json+zstd work each run.
    _json = nc.to_json_bytes()
    nc.to_json_bytes = lambda: _json
    return nc


class _Runner:
    """Persistent shard_map'd bass_exec executable (the axon redirect path
    of run_bass_kernel_spmd, with the jit built ONCE and reused so the NEFF
    stays loaded across calls)."""

    def __init__(self, nc, ncores):
        bass2jax.install_neuronx_cc_hook()
        self.nc, self.ncores = nc, ncores
        partition_name = (nc.partition_id_tensor.name
                          if nc.partition_id_tensor else None)
        in_names, out_names, out_avals, zero_outs = [], [], [], []
        for alloc in nc.m.functions[0].allocations:
            if not isinstance(alloc, mybir.MemoryLocationSet):
                continue
            name = alloc.memorylocations[0].name
            if alloc.kind == 'ExternalInput':
                if name != partition_name:
                    in_names.append(name)
            elif alloc.kind == 'ExternalOutput':
                out_names.append(name)
                shape = tuple(alloc.tensor_shape)
                dtype = mybir.dt.np(alloc.dtype)
                out_avals.append(jax.core.ShapedArray(shape, dtype))
                zero_outs.append((shape, dtype))
        assert nc.dbg_addr is None
        n_params = len(in_names)
        in_names_all = in_names + out_names
        if partition_name is not None:
            in_names_all.append(partition_name)
        donate = tuple(range(n_params, n_params + len(out_avals)))
        self.in_names, self.out_names = in_names, out_names
        self.out_avals, self.zero_outs = out_avals, zero_outs

        def _body(*args):
            operands = list(args)
            if partition_name is not None:
                operands.append(partition_id_tensor())
            outs = _bass_exec_p.bind(
                *operands, out_avals=tuple(out_avals),
                in_names=tuple(in_names_all), out_names=tuple(out_names),
                lowering_input_output_aliases=(), sim_require_finite=True,
                sim_require_nnan=True, nc=nc)
            return tuple(outs)

        devices = jax.devices()[:ncores]
        assert len(devices) == ncores
        mesh = Mesh(np.asarray(devices), ('core',))
        nio = n_params + len(out_avals)
        self.sharded = jax.jit(
            shard_map(_body, mesh=mesh,
                      in_specs=(PartitionSpec('core'),) * nio,
                      out_specs=(PartitionSpec('core'),) * len(out_names),
                      check_rep=False),
            donate_argnums=donate, keep_unused=True)

    def __call__(self, concat_in):
        zeros = [np.zeros((self.ncores * s[0], *s[1:]), d)
                 for s, d in self.zero_outs]
        outs = self.sharded(*concat_in, *zeros)
        return {name: np.asarray(o) for name, o in zip(self.out_names, outs)}


def prepare_all(cfg, x, edge_index, W1, att_src1, att_dst1, b1,
                W2, att_src2, att_dst2, b2, W3, att_src3, att_dst3, b3):
    # self-loops (PyG add_self_loops) are handled analytically in the
    # normalize/final phases from the local h_own rows, so they are NOT
    # materialized as edges: smaller tables and less gather padding skew.
    src = np.asarray(edge_index[0]).astype(np.int64)
    dst = np.asarray(edge_index[1]).astype(np.int64)
    NP, per_core = _prepare_edges(cfg, src, dst)
    W1e_ = _ext_w(np.asarray(W1, np.float32), np.asarray(att_src1, np.float32),
                  np.asarray(att_dst1, np.float32))
    W2e_ = _ext_w(np.asarray(W2, np.float32), np.asarray(att_src2, np.float32),
                  np.asarray(att_dst2, np.float32))
    W3e_ = _ext_w(np.asarray(W3, np.float32), np.asarray(att_src3, np.float32),
                  np.asarray(att_dst3, np.float32))
    x = np.asarray(x, np.float32)
    wb_ = np.ascontiguousarray(np.concatenate([W1e_, W2e_, W3e_], axis=1))
    wblob = np.zeros(8 * cfg.WSLI, np.int16)
    wblob[0:cfg.WLEN] = wb_.view(np.int16).ravel()
    wblob[cfg.WLEN:cfg.WLEN + cfg.BLEN] = np.concatenate(
        [np.asarray(b1).ravel(), np.asarray(b2).ravel(),
         np.asarray(b3).ravel()]).astype(np.float32).view(np.int16)
    wblob[cfg.WLEN + cfg.BLEN:cfg.WLEN + cfg.BLEN + 2 * cfg.F] = (
        np.concatenate([np.asarray(att_src1, np.float32).ravel(),
                        np.asarray(att_src2, np.float32).ravel()])
        .astype(BF).view(np.int16))
    in_maps = []
    lvl = 63 if cfg.XBITS == 7 else 127
    for c in range(cfg.NCORES):
        xc = x[c * cfg.NPC:(c + 1) * cfg.NPC]
        sc = np.maximum(np.abs(xc).max(axis=1), 1e-30) / lvl     # [NPC]
        xq = np.clip(np.round(xc / sc[:, None]), -lvl, lvl).astype(np.int8)
        scp = np.ones(cfg.NPCP, np.float32)
        scp[:cfg.NPC] = sc
        xs_flat = scp.reshape(cfg.NT, 128).T.astype(np.float32).ravel()
        if cfg.XBITS == 7:
            up = np.full((cfg.NPCP, cfg.F), 64, np.uint8)   # pads: v=0
            up[:cfg.NPC] = (xq.astype(np.int16) + 64).astype(np.uint8)
            ut = np.ascontiguousarray(up.T).reshape(cfg.F, cfg.NPCP // 8, 8)
            bits = ((ut[:, :, 7:] >> np.arange(7)) & 1).astype(np.uint8)
            xpart = np.ascontiguousarray(
                ut[:, :, 0:7] | (bits << 7)).ravel().view(np.int16)
        else:
            xpart = np.ascontiguousarray(xq.T).ravel().view(np.int16)
        wchunk = (wblob[c * cfg.WSLI:(c + 1) * cfg.WSLI] if cfg.WAG
                  else wblob)
        blob = np.concatenate([
            per_core[c]['ei'].ravel(),
            xpart,
            wchunk,
            xs_flat.view(np.int16)])
        in_maps.append(np.ascontiguousarray(blob)[None, :])
    return NP, in_maps


_CACHE = {}
LAST_RESULT = None
LAST_RUN = None


def run_again():
    import time
    runner, concat_in, cfg = LAST_RUN
    t0 = time.monotonic()
    runner(concat_in)
    return time.monotonic() - t0


def kernel(x, edge_index, W1, att_src1, att_dst1, b1, W2, att_src2, att_dst2,
           b2, W3, att_src3, att_dst3, b3):
    global LAST_RESULT, LAST_RUN
    x = np.asarray(x)
    edge_index = np.asarray(edge_index)
    cfg = Cfg(N=x.shape[0], F=x.shape[1], H=np.asarray(att_src1).shape[0],
              C=np.asarray(att_src1).shape[1], OUT=np.asarray(W3).shape[1])
    NP, in_maps = prepare_all(cfg, x, edge_index, W1, att_src1, att_dst1,
                              b1, W2, att_src2, att_dst2, b2, W3, att_src3,
                              att_dst3, b3)
    key = (cfg.N, cfg.F, NP)
    if key not in _CACHE:
        nc = _build(cfg)
        _CACHE[key] = _Runner(nc, cfg.NCORES)
    runner = _CACHE[key]
    concat_in = [np.concatenate(in_maps, axis=0)]
    LAST_RUN = (runner, concat_in, cfg)
    res = runner(concat_in)
    LAST_RESULT = res
    oo = res['out_own'].reshape(cfg.NCORES, cfg.NPCP, cfg.OUT)
    out = np.concatenate([oo[c][:cfg.NPC] for c in range(cfg.NCORES)], axis=0)
    return out.astype(np.float32)
